# revision 6
# baseline (speedup 1.0000x reference)
"""Trainium2 Bass kernel for nn_Attention_4088808866132 (topk_masking).

Data-parallel over batch B=16 across 8 NeuronCores (2 batches/core).

Algebraic restructuring vs the reference:
  - Pass-1 MHA output is discarded; only head-averaged attention weights are
    needed. With a single query token the K-projection folds into the query:
        scores[b,n,h] = v_seq[b,n,:] . rq[b,h,:],   rq = (Wk_h^T qh_h)/sqrt(hd)
  - top_k(softmax(x)) == top_k(x), and attention is permutation-invariant over
    keys, so only the top-K *set* matters; pass 2 is a masked softmax over
    pass-1 scores. Host does the tiny O(E^2) projections.

v3 (cost-model v1 driven): the kernel is DMA-queue bound (3 queues: SP/Act/
Pool at ~332 GB/s each, per-queue serial).  Changes vs v2:
  - Loads emitted in strict phase order (vT b0, vT b1, v b0, v b1) and
    greedily balanced across the three queues so the last load lands ASAP.
  - B-phase runs token-major, consuming v chunks as they arrive, so only the
    final chunk's matmuls remain after the last load.
  - Act's Exp table (1283ns) is paid at t=0 via a dummy activation; exps are
    inserted between Act's DMA chunks at the point they become ready.
  - rq+nst packed into one input DMA; t/z2 for both batches merged into one
    output tensor written by a single tail DMA.
"""

import numpy as np

B, N, E, H = 16, 4096, 1024, 16
HD = E // H
K = 2048
NCORES = 8
BPC = B // NCORES          # batches per core
NT = N // 128              # 32 n-tiles per batch
EC = E // 128              # 8 e-chunks
NVT = 8                    # vT chunks per batch (n-span 512)
NV = 4                     # v chunks per batch (8 token-tiles, 1MB)
QSC = 16.0                 # rq prescale so fp8 cast stays in normal range
RNG = 0.006                # threshold search half-window around host center
NROUNDS = 2
NTAU = 8                   # taus per round (9-ary search; 81 cells on 2*RNG)
OUTW = 144                 # out cols per batch: 128 tT + 16 z2


def build_bass():
    import concourse.mybir as mybir
    from concourse import bacc
    from concourse.tile import TileContext

    dt = mybir.dt
    AF = mybir.ActivationFunctionType
    OP = mybir.AluOpType
    AX = mybir.AxisListType

    nc = bacc.Bacc()
    global PHASE_MARKS
    PHASE_MARKS = []

    def mark(label):
        PHASE_MARKS.append((label, int(nc.next_id())))

    v_ext = nc.dram_tensor("v8", (BPC, N, E), dt.float8e4, kind="ExternalInput")
    vt_ext = nc.dram_tensor("vT8", (BPC, E, N), dt.float8e4,
                            kind="ExternalInput")
    # packed small input: cols 0:64 = rq fp8 bytes (128 x 256 f8 as f32),
    # cols 64:130 = nst (BPC x (NT+1) noise/lo cols)
    pk_ext = nc.dram_tensor("pk", (128, 130), dt.float32, kind="ExternalInput")
    to_ext = nc.dram_tensor("to", (BPC, 128, OUTW), dt.float32,
                            kind="ExternalOutput")

    # ---- static queue schedule (cost-model v1: ~3.158 ns/KB per queue) ----
    CB = 0.0030116  # ns per byte on a DMA queue
    chunks = ([("vt", b, q) for b in range(BPC) for q in range(NVT)]
              + [("v", b, j) for b in range(BPC) for j in range(NV)])
    ccost = {"vt": int(512 * 1024 * CB), "v": int(1024 * 1024 * CB)}
    qnames = ("sp", "act", "pool")
    clock = {"sp": 0.0, "act": 1283.0, "pool": 802.0}  # act: table; pool: pk
    streams = {q: [] for q in qnames}
    ends = {}
    for ch in chunks:
        qn = min(qnames, key=lambda q: clock[q])
        streams[qn].append(ch)
        clock[qn] += ccost[ch[0]]
        ends[ch] = clock[qn]
    # predicted completion of each load phase
    vt_done = {b: max(ends[c] for c in chunks if c[0] == "vt" and c[1] == b)
               for b in range(BPC)}
    # insert exp(b) into act's stream after the chunk whose end covers SC(b)
    for b in range(BPC):
        t, pos = 1283.0, 0
        for i, ch in enumerate(streams["act"]):
            if t >= vt_done[b] + 150:
                break
            t += ccost.get(ch[0], 612)  # prior exp items cost ~612ns
            pos = i + 1
        streams["act"].insert(pos, ("exp", b))

    with TileContext(nc) as tc:
        with (
            tc.tile_pool(name="const", bufs=1) as cpool,
            tc.tile_pool(name="vbuf", bufs=1) as vpool,
            tc.tile_pool(name="vtbuf", bufs=1) as vtpool,
            tc.tile_pool(name="e2p", bufs=2) as e2pool,
            tc.tile_pool(name="gep", bufs=2) as gepool,
            tc.tile_pool(name="w2p", bufs=2) as w2pool,
            tc.tile_pool(name="small", bufs=2) as smpool,
            tc.tile_pool(name="outp", bufs=1) as opool,
            tc.tile_pool(name="scps", bufs=2, space="PSUM") as scps,
            tc.tile_pool(name="bps", bufs=2, space="PSUM") as bps,
            tc.tile_pool(name="smps", bufs=2, space="PSUM") as smps,
            tc.tile_pool(name="brps", bufs=2, space="PSUM") as brps,
        ):
            # ---- constants ----
            ones_f32 = cpool.tile([128, 1], dt.float32)
            nc.vector.memset(ones_f32, 1.0)
            ones_f8 = cpool.tile([128, 1], dt.float8e4)
            nc.vector.memset(ones_f8, 1.0)
            ones_row = cpool.tile([1, 128], dt.float32)
            nc.vector.memset(ones_row, 1.0)
            kvec = cpool.tile([128, NTAU], dt.float32)
            for k in range(NTAU):
                nc.vector.memset(kvec[:, k:k + 1], float(k + 1))

            # merged output tile for both batches; doubles as the B-phase
            # accumulator (chunk partials added in), so zero it all upfront.
            out_sb = opool.tile([128, BPC * OUTW], dt.float32)
            nc.vector.memset(out_sb, 0.0)

            # dummy activation to pay the Exp table load at t=0
            dumm = cpool.tile([1, 8], dt.float32)
            nc.vector.memset(dumm, 0.0)

            # ---- packed small load first on Pool ----
            pk_sb = cpool.tile([128, 130], dt.float32)
            nc.gpsimd.dma_start(out=pk_sb, in_=pk_ext[:])
            rq_sb = pk_sb[:, 0:64].bitcast(dt.float8e4)   # [128, 256]
            nst_sb = pk_sb[:, 64:130]                      # [128, 66]

            v_sb, vt_sb = {}, {}
            for b in range(BPC):
                vt_sb[b] = vtpool.tile([128, EC * N], dt.float8e4,
                                       tag=f"vt{b}", name=f"vt{b}")
                v_sb[b] = vpool.tile([128, NT * E], dt.float8e4,
                                     tag=f"v{b}", name=f"v{b}")

            def emit_chunk(eng, ch):
                kind, b, q = ch
                if kind == "vt":
                    out = vt_sb[b].rearrange("p (c n) -> p c n", c=EC)[
                        :, :, q * 512:(q + 1) * 512]
                    in_ = vt_ext[b].rearrange("(c p) n -> p c n", p=128)[
                        :, :, q * 512:(q + 1) * 512]
                else:
                    out = v_sb[b][:, 8 * q * E:(8 * q + 8) * E].rearrange(
                        "p (t e) -> p t e", t=8)
                    in_ = v_ext[b, 8 * q * 128:(8 * q + 8) * 128, :].rearrange(
                        "(t p) e -> p t e", p=128)
                eng.dma_start(out=out, in_=in_)

            # ================ per-batch state ================
            st = {b: {} for b in range(BPC)}

            def sc_group(b, q):  # SC for vt chunk q: token tiles 4q..4q+3
                if q == 0:
                    mark(f"b{b}:SC")
                    st[b]["sc"] = scps.tile([128, NT * H], dt.float32,
                                            tag="sc", name=f"sc{b}")
                sc = st[b]["sc"]
                vt_v = vt_sb[b].rearrange("p (c n) -> p c n", c=EC)
                rq_v = rq_sb.rearrange("p (b c h) -> p b c h", b=BPC, c=EC)
                for j in range(4):
                    t = 4 * q + j
                    lhs = vt_v[:, :, t * 128:(t + 1) * 128]
                    for c2 in range(EC // 2):
                        nc.tensor.matmul(
                            sc[:, t * H:(t + 1) * H],
                            lhs[:, 2 * c2:2 * c2 + 2, :],
                            rq_v[:, b, 2 * c2:2 * c2 + 2, :],
                            start=(c2 == 0), stop=(c2 == EC // 2 - 1),
                            perf_mode=mybir.MatmulPerfMode.DoubleRow)

            def exp_phase(b):
                mark(f"b{b}:X")
                E2 = e2pool.tile([128, NT * H], dt.float32, tag="E2",
                                 name=f"E2_{b}")
                nc.scalar.activation(out=E2, in_=st[b]["sc"], func=AF.Exp,
                                     scale=1.0 / QSC)
                st[b]["E2"] = E2

            def z1red_phase(b):
                E2r = smpool.tile([128, H], dt.float32, tag="E2r",
                                  name=f"E2r{b}")
                nc.vector.tensor_reduce(
                    out=E2r, in_=st[b]["E2"].rearrange("p (t h) -> p h t", t=NT),
                    axis=AX.X, op=OP.add)
                st[b]["E2r"] = E2r

            def z1mm_phase(b):
                z1p = smps.tile([1, H], dt.float32, tag="acc", name=f"z1p{b}")
                nc.tensor.matmul(z1p, ones_f32, st[b]["E2r"],
                                 start=True, stop=True)
                st[b]["z1p"] = z1p

            def w16_phase(b):
                w16 = smpool.tile([1, H], dt.float32, tag="w16",
                                  name=f"w16_{b}")
                nc.vector.tensor_scalar(
                    out=w16, in0=st[b]["z1p"], scalar1=float(H), scalar2=None,
                    op0=OP.mult)
                nc.vector.reciprocal(w16, w16)
                st[b]["w16"] = w16

            def wrep_phase(b):  # PE rank-1 broadcast into psum
                wrep = brps.tile([128, H], dt.float32, tag="bc",
                                 name=f"wrep{b}")
                nc.tensor.matmul(wrep, ones_row, st[b]["w16"],
                                 start=True, stop=True)
                st[b]["wrep"] = wrep

            def noisy_phase(b):
                mark(f"b{b}:W")
                E2 = st[b]["E2"]
                awt = e2pool.tile([128, NT * H], dt.float32, tag="awt",
                                  name=f"awt{b}")
                nc.vector.tensor_tensor(
                    out=awt.rearrange("p (t h) -> p t h", t=NT),
                    in0=E2.rearrange("p (t h) -> p t h", t=NT),
                    in1=st[b]["wrep"].unsqueeze(1).to_broadcast([128, NT, H]),
                    op=OP.mult)
                noisy = smpool.tile([128, NT], dt.float32, tag="noisy",
                                    name=f"noisy{b}")
                nc.vector.tensor_reduce(
                    out=noisy, in_=awt.rearrange("p (t h) -> p t h", t=NT),
                    axis=AX.X, op=OP.add)
                nc.vector.tensor_tensor(
                    out=noisy, in0=noisy,
                    in1=nst_sb[:, b * (NT + 1):b * (NT + 1) + NT], op=OP.add)
                st[b]["noisy"] = noisy

            def search_init(b):
                mark(f"b{b}:S")
                lo = smpool.tile([128, 1], dt.float32, tag="lo", name=f"lo{b}")
                nc.vector.tensor_copy(
                    out=lo, in_=nst_sb[:, b * (NT + 1) + NT:(b + 1) * (NT + 1)])
                stp = smpool.tile([128, 1], dt.float32, tag="stp",
                                  name=f"stp{b}")
                nc.vector.memset(stp, 2.0 * RNG / (NTAU + 1.0))
                st[b]["lo"], st[b]["stp"] = lo, stp

            def search_ge(b, r):
                lo, stp = st[b]["lo"], st[b]["stp"]
                taus = smpool.tile([128, NTAU], dt.float32, tag="taus",
                                   name=f"taus{b}_{r}")
                nc.vector.tensor_scalar(
                    out=taus, in0=kvec[:, 0:NTAU], scalar1=stp, scalar2=lo,
                    op0=OP.mult, op1=OP.add)
                ge = gepool.tile([128, NTAU * NT], dt.float8e4, tag="ge",
                                 name=f"ge{b}_{r}")
                nc.vector.tensor_tensor(
                    out=ge.rearrange("p (k t) -> p k t", k=NTAU),
                    in0=st[b]["noisy"].unsqueeze(1).to_broadcast([128, NTAU, NT]),
                    in1=taus.unsqueeze(2).to_broadcast([128, NTAU, NT]),
                    op=OP.is_ge)
                st[b]["ge"] = ge

            def search_cnt(b, r):  # PE: count + broadcast partials
                ge = st[b]["ge"]
                cnt = smps.tile([1, NTAU], dt.float32, tag="acc",
                                name=f"cnt{b}_{r}")
                gev = ge.rearrange("p (k t) -> p t k", k=NTAU)
                for t in range(NT):
                    nc.tensor.matmul(cnt, ones_f8, gev[:, t, :],
                                     start=(t == 0), stop=(t == NT - 1))
                st[b]["cnt"] = cnt

            def search_carrow(b, r):
                carrow = smpool.tile([1, NTAU], dt.float32, tag="carrow",
                                     name=f"car_{b}_{r}")
                nc.vector.tensor_copy(out=carrow, in_=st[b]["cnt"])
                st[b]["carrow"] = carrow

            def search_carbc(b, r):  # PE broadcast counts to all partitions
                car = brps.tile([128, NTAU], dt.float32, tag="bc",
                                name=f"carb{b}_{r}")
                nc.tensor.matmul(car, ones_row, st[b]["carrow"],
                                 start=True, stop=True)
                st[b]["car"] = car

            def search_update(b, r):
                lo, stp = st[b]["lo"], st[b]["stp"]
                geK = smpool.tile([128, NTAU], dt.float32, tag="geK",
                                  name=f"geK{b}_{r}")
                mm = smpool.tile([128, 1], dt.float32, tag="mm",
                                 name=f"mm{b}_{r}")
                nc.vector.tensor_scalar(
                    out=geK, in0=st[b]["car"], scalar1=float(K), scalar2=0.0,
                    op0=OP.is_ge, op1=OP.add, accum_out=mm)
                nc.vector.tensor_scalar(
                    out=lo, in0=mm, scalar1=stp, scalar2=lo,
                    op0=OP.mult, op1=OP.add)
                if r != NROUNDS - 1:
                    nc.vector.tensor_scalar(
                        out=stp, in0=stp, scalar1=1.0 / (NTAU + 1.0),
                        scalar2=None, op0=OP.mult)

            def w2_phase(b):
                mark(f"b{b}:M")
                w2 = w2pool.tile([128, NT * H], dt.float8e4, tag="w2",
                                 name=f"w2_{b}")
                nc.vector.scalar_tensor_tensor(
                    out=w2.rearrange("p (t h) -> p t h", t=NT),
                    in0=st[b]["noisy"].unsqueeze(2).to_broadcast([128, NT, H]),
                    scalar=st[b]["lo"],
                    in1=st[b]["E2"].rearrange("p (t h) -> p t h", t=NT),
                    op0=OP.is_ge, op1=OP.mult)
                st[b]["w2"] = w2

            def z2_phase(b):
                z2p = smps.tile([1, H], dt.float32, tag="acc", name=f"z2p{b}")
                w2 = st[b]["w2"]
                for t in range(NT):
                    nc.tensor.matmul(z2p, ones_f8, w2[:, t * H:(t + 1) * H],
                                     start=(t == 0), stop=(t == NT - 1))
                st[b]["z2p"] = z2p

            def b_group(b, j):  # B-phase for v chunk j: token tiles 8j..8j+7
                # Closed psum groups per (chunk, c); the chunk partial is then
                # DVE-added into out_sb, which accumulates across chunks.
                if j == 0:
                    mark(f"b{b}:B")
                    st[b]["tTp"] = bps.tile([128, EC * H], dt.float32,
                                            tag="tT", name=f"tTp{b}")
                tTp, w2 = st[b]["tTp"], st[b]["w2"]
                v_v = v_sb[b].rearrange("p (t e) -> p t e", t=NT)
                w2v = w2.rearrange("p (t h) -> p t h", t=NT)
                for c in range(EC):
                    for s in range(4):
                        t0 = 8 * j + 2 * s
                        nc.tensor.matmul(
                            tTp[:, c * H:(c + 1) * H],
                            v_v[:, t0:t0 + 2, c * 128:(c + 1) * 128],
                            w2v[:, t0:t0 + 2, :],
                            start=(s == 0), stop=(s == 3),
                            perf_mode=mybir.MatmulPerfMode.DoubleRow)
                acc = out_sb[:, b * OUTW:b * OUTW + 128]
                nc.vector.tensor_tensor(out=acc, in0=acc, in1=tTp, op=OP.add)

            def out_copy(b):
                mark(f"b{b}:O")
                nc.vector.tensor_copy(
                    out=out_sb[0:1, b * OUTW + 128:b * OUTW + 128 + H],
                    in_=st[b]["z2p"])

            # ================ emission ================
            mark("L")
            # Act: dummy exp first (pays table load at t=0)
            nc.scalar.activation(out=dumm, in_=dumm, func=AF.Exp, scale=1.0)

            # DMA streams; Act's stream is split at its exp markers so the
            # exps can be emitted after the SC matmuls they read.
            engs = {"sp": nc.sync, "act": nc.scalar, "pool": nc.gpsimd}
            for qn in ("sp", "pool"):
                for ch in streams[qn]:
                    emit_chunk(engs[qn], ch)
            act_segs = [[]]
            for ch in streams["act"]:
                if ch[0] == "exp":
                    act_segs.append([])
                else:
                    act_segs[-1].append(ch)
            while len(act_segs) < 3:
                act_segs.append([])

            for ch in act_segs[0]:
                emit_chunk(nc.scalar, ch)
            for q in range(NVT):
                sc_group(0, q)
            exp_phase(0)
            for ch in act_segs[1]:
                emit_chunk(nc.scalar, ch)
            z1red_phase(0)
            z1mm_phase(0)
            w16_phase(0)
            wrep_phase(0)
            for q in range(0, 4):
                sc_group(1, q)
            noisy_phase(0)
            search_init(0)
            search_ge(0, 0)
            search_cnt(0, 0)
            search_carrow(0, 0)
            search_carbc(0, 0)
            search_update(0, 0)
            search_ge(0, 1)
            sc_group(1, 4)
            sc_group(1, 5)
            search_cnt(0, 1)
            search_carrow(0, 1)
            search_carbc(0, 1)
            search_update(0, 1)
            w2_phase(0)
            sc_group(1, 6)
            sc_group(1, 7)
            exp_phase(1)
            for ch in act_segs[2]:
                emit_chunk(nc.scalar, ch)
            z1red_phase(1)
            z1mm_phase(1)
            w16_phase(1)
            wrep_phase(1)
            z2_phase(0)
            noisy_phase(1)
            search_init(1)
            search_ge(1, 0)
            b_group(0, 0)
            search_cnt(1, 0)
            search_carrow(1, 0)
            search_carbc(1, 0)
            search_update(1, 0)
            search_ge(1, 1)
            b_group(0, 1)
            search_cnt(1, 1)
            search_carrow(1, 1)
            search_carbc(1, 1)
            search_update(1, 1)
            w2_phase(1)
            b_group(0, 2)
            z2_phase(1)
            b_group(0, 3)
            out_copy(0)
            for j in range(NV):
                b_group(1, j)
            out_copy(1)
            nc.sync.dma_start(
                out=to_ext[:],
                in_=out_sb.rearrange("p (b f) -> p b f", b=BPC))

    nc.finalize()
    return nc


_NC_CACHE = None
LAST_EXEC_NS = None
PHASE_MARKS = []


def host_prep(v_seq, q_global, noise, in_proj_w, in_proj_b):
    """Host-side layout/dtype prep shared by kernel() and the test harness."""
    import ml_dtypes
    f8 = ml_dtypes.float8_e4m3fn

    Wq, Wk = in_proj_w[:E], in_proj_w[E:2 * E]
    bq = in_proj_b[:E]

    v8 = np.asarray(v_seq, np.float32).astype(f8)            # (B,N,E)
    vT8 = np.ascontiguousarray(v8.transpose(0, 2, 1))        # (B,E,N)

    qh = (q_global @ Wq.T + bq).reshape(B, H, HD)
    scale = 1.0 / np.sqrt(HD)
    rq = np.einsum('bhd,hde->bhe', qh, Wk.reshape(H, HD, E)) * (scale * QSC)
    rqt8 = np.ascontiguousarray(rq.transpose(0, 2, 1)).astype(f8)  # (B,E,H)

    noise = np.asarray(noise, np.float32)
    nstv = (noise * 0.05).reshape(B, NT, 128).transpose(0, 2, 1)  # (B,128,NT)
    lo0 = (1.0 / N + 0.05 * noise.mean(axis=1) - RNG).astype(np.float32)  # (B,)

    in_maps = []
    for core in range(NCORES):
        sl = slice(core * BPC, (core + 1) * BPC)
        rq_core = rqt8[sl].reshape(BPC, EC, 128, H).transpose(2, 0, 1, 3)
        nst_core = np.empty((128, BPC, NT + 1), np.float32)
        nst_core[:, :, :NT] = nstv[sl].transpose(1, 0, 2)
        nst_core[:, :, NT] = lo0[sl][None, :]
        pk = np.empty((128, 130), np.float32)
        pk[:, 0:64] = np.ascontiguousarray(
            rq_core.reshape(128, BPC * EC * H)).view(np.float32)
        pk[:, 64:130] = nst_core.reshape(128, BPC * (NT + 1))
        in_maps.append({
            "v8": np.ascontiguousarray(v8[sl]),
            "vT8": np.ascontiguousarray(vT8[sl]),
            "pk": pk,
        })
    return in_maps


def kernel(v_seq, v_global, q_seq, q_global, noise,
           in_proj_w, in_proj_b, out_proj_w, out_proj_b):
    global _NC_CACHE, LAST_EXEC_NS
    from concourse.bass_utils import run_bass_kernel_spmd

    q_global = np.asarray(q_global, np.float32)
    in_proj_w = np.asarray(in_proj_w, np.float32)
    in_proj_b = np.asarray(in_proj_b, np.float32)
    out_proj_w = np.asarray(out_proj_w, np.float32)
    out_proj_b = np.asarray(out_proj_b, np.float32)

    Wv = in_proj_w[2 * E:]
    bv = in_proj_b[2 * E:]

    in_maps = host_prep(v_seq, q_global, noise, in_proj_w, in_proj_b)

    if _NC_CACHE is None:
        _NC_CACHE = build_bass()
    nc = _NC_CACHE

    import os
    trace = bool(int(os.environ.get("KTRACE", "0")))
    res = run_bass_kernel_spmd(nc, in_maps, core_ids=list(range(NCORES)),
                               trace=trace)
    LAST_EXEC_NS = getattr(res, "exec_time_ns", None)
    outs = res.results

    to = np.concatenate([np.asarray(outs[c]["to"]) for c in range(NCORES)], 0)
    tT = to[:, :, 0:128]                   # (B, 128, EC*H)
    z2 = to[:, 0, 128:128 + H]             # (B, H)
    # tT: [b, i, c*H+h] = t[b, h, c*128+i]
    t_dev = tT.reshape(B, 128, EC, H).transpose(0, 3, 2, 1).reshape(B, H, E)
    z_dev = z2.reshape(B, H)

    ctx = np.einsum('hde,bhe->bhd', Wv.reshape(H, HD, E),
                    t_dev / z_dev[..., None]) + bv.reshape(H, HD)[None]
    att = ctx.reshape(B, E) @ out_proj_w.T + out_proj_b
    return np.concatenate([att, q_global], axis=1)
